# revision 1
# baseline (speedup 1.0000x reference)
"""DynamicSparseMoE Trainium2 kernel.

Math (per token t):
  logits[e'] = x[t] . gate_w[e'] + gate_b[e']        (C=2048 contraction)
  gw[e']     = 1.0 if logits[e'] > 0 else 0.0
  expert e input: xe[d] = x[t, 16*d + e]  (d=0..127; expert idx fastest in channel)
  h  = gelu(fc_w[e] @ xe + fc_b[e])                   (H=512)
  oe = proj_w[e] @ h + proj_b[e]                      (DE=128)
  out[t, 128*e + d] = gw[e] * oe[d]                   (expert-major output channels)

Strategy: data-parallel over the 16384 tokens across 8 NeuronCores (2048
tokens/core).  Per 512-token group:
  pass1 (per expert): 4 PE transposes of the stride-16 channel slice of the
    row-major x tile -> xe^T [de, tok] fp32; DVE evacuation; 4 exact-fp32
    gate matmuls (activation-stationary, slice-accumulated into a single
    PSUM bank); GPSIMD cast to fp32r; fc as fp32r matmuls (weights
    stationary, N=512); gelu+fc_bias fused on ACT writing fp32r; proj as
    fp32r matmuls accumulating K=512; proj_bias fused into the PSUM
    evacuation (bf16 out).
  pass2: gate threshold (is_gt) on DVE.
  pass3 (per expert): 4 bf16 PE exit transposes; gate multiply fused into
    the final PSUM->SBUF copy as a per-token tensor_scalar; contiguous
    row stores.
"""

import sys

for _p in ("/opt/trn_rl_repo", "/root/.axon_site"):
    if _p not in sys.path:
        sys.path.insert(0, _p)

import ml_dtypes
import numpy as np

import concourse.mybir as mybir
from concourse import bacc
from concourse.bass_utils import run_bass_kernel_spmd
from concourse.tile import TileContext


B, T, C, E = 8, 2048, 2048, 16
DE = C // E  # 128
H = 4 * DE  # 512
NCORES = 8
NTOK = B * T  # 16384
TPC = NTOK // NCORES  # tokens per core: 2048
GROUP = 512  # tokens per group
NTAU = GROUP // 128  # 4 token-tiles per group
NGRP = TPC // GROUP  # 4 groups per core

F32 = mybir.dt.float32
F32R = mybir.dt.float32r
BF16 = mybir.dt.bfloat16
AF = mybir.ActivationFunctionType
ALU = mybir.AluOpType
GELU = AF.Gelu

_CACHE = {}


def _build():
    nc = bacc.Bacc(trn_type="TRN2", num_devices=NCORES)

    x_d = nc.dram_tensor("x", [TPC, C], F32, kind="ExternalInput").ap()
    gwp_d = nc.dram_tensor("gwp", [C, E], F32, kind="ExternalInput").ap()
    fcw_d = nc.dram_tensor("fcw", [E, DE, H], F32, kind="ExternalInput").ap()
    pjw_d = nc.dram_tensor("pjw", [E, 4, 128, DE], F32, kind="ExternalInput").ap()
    fcb_d = nc.dram_tensor("fcb", [128, 64], F32, kind="ExternalInput").ap()
    pjb_d = nc.dram_tensor("pjb", [128, E], F32, kind="ExternalInput").ap()
    ngb_d = nc.dram_tensor("ngb", [128, E], F32, kind="ExternalInput").ap()
    idn_d = nc.dram_tensor("idn", [128, 128], F32, kind="ExternalInput").ap()
    idnb_d = nc.dram_tensor("idnb", [128, 128], BF16, kind="ExternalInput").ap()
    out_d = nc.dram_tensor("out", [TPC, C], F32, kind="ExternalOutput").ap()

    with TileContext(nc) as tc:
        with (
            tc.tile_pool(name="wts", bufs=1) as wts,
            tc.tile_pool(name="work", bufs=2) as work,
            tc.tile_pool(name="psum", bufs=2, space="PSUM") as psum,
        ):
            # ---- resident weights ----
            gwp_sb = wts.tile([128, E * E], F32)  # [p, chunk*16+e']
            nc.sync.dma_start(
                out=gwp_sb.rearrange("p (k e) -> p k e", k=E),
                in_=gwp_d.rearrange("(k p) e -> p k e", p=128),
            )
            # fc/proj weights: SWDGE dma with cast fp32 -> fp32r
            fcw_sb = wts.tile([128, E * H], F32R)  # [de, e*512+h]
            nc.gpsimd.dma_start(
                out=fcw_sb.rearrange("p (e h) -> p e h", e=E),
                in_=fcw_d.rearrange("e p h -> p e h"),
            )
            pjw_sb = wts.tile([128, E * 4 * DE], F32R)  # [h_in_chunk, (e*4+q)*128+d]
            nc.gpsimd.dma_start(
                out=pjw_sb.rearrange("p (e q d) -> p e q d", e=E, q=4),
                in_=pjw_d.rearrange("e q p d -> p e q d"),
            )
            fcb_sb = wts.tile([128, 64], F32)
            nc.sync.dma_start(out=fcb_sb, in_=fcb_d)
            pjb_sb = wts.tile([128, E], F32)
            nc.sync.dma_start(out=pjb_sb, in_=pjb_d)
            ngb_sb = wts.tile([128, E], F32)
            nc.sync.dma_start(out=ngb_sb, in_=ngb_d)
            idn_sb = wts.tile([128, 128], F32)
            nc.sync.dma_start(out=idn_sb, in_=idn_d)
            idnb_sb = wts.tile([128, 128], BF16)
            nc.sync.dma_start(out=idnb_sb, in_=idnb_d)

            for g in range(NGRP):
                t0 = g * GROUP
                xrow = []
                for ti in range(NTAU):
                    xt = work.tile([128, C], F32, tag="xrow", bufs=4)
                    nc.sync.dma_start(
                        out=xt, in_=x_d[t0 + ti * 128 : t0 + (ti + 1) * 128, :]
                    )
                    xrow.append(xt)

                ps_g = psum.tile([16, GROUP], F32, tag="gate", bufs=1)
                xpTr = []
                pjT = []
                # ---- pass 1: per-expert transposes, gate, fc, gelu, proj ----
                for e in range(E):
                    ps_t = psum.tile([128, GROUP], F32, tag="tp", bufs=3)
                    for ti in range(NTAU):
                        lhs = xrow[ti].rearrange("p (d e) -> p e d", e=E)[:, e, :]
                        nc.tensor.transpose(
                            ps_t[:, ti * 128 : (ti + 1) * 128], lhs, idn_sb
                        )
                    xe = work.tile([128, GROUP], F32, tag="xpT", bufs=4)
                    nc.vector.tensor_copy(xe, ps_t)
                    # gate: exact fp32, weights stationary (tiny LDW), one bank
                    nc.tensor.matmul(
                        ps_g,
                        lhsT=gwp_sb[:, e * E : (e + 1) * E],
                        rhs=xe,
                        start=(e == 0),
                        stop=(e == E - 1),
                    )
                    xer = work.tile([128, GROUP], F32R, tag="xpTr", bufs=3)
                    nc.vector.tensor_copy(xer, xe)
                    xpTr.append(xer)

                    h_sb = work.tile([128, 4 * GROUP], F32R, tag="h", bufs=3)
                    for hq in range(4):
                        ps_fc = psum.tile([128, GROUP], F32, tag="fc", bufs=2)
                        nc.tensor.matmul(
                            ps_fc,
                            lhsT=fcw_sb[:, e * H + hq * 128 : e * H + (hq + 1) * 128],
                            rhs=xer,
                            start=True,
                            stop=True,
                        )
                        nc.scalar.activation(
                            h_sb[:, hq * GROUP : (hq + 1) * GROUP],
                            ps_fc,
                            GELU,
                            bias=fcb_sb[:, e * 4 + hq : e * 4 + hq + 1],
                            scale=1.0,
                        )
                    ps_pj = psum.tile([128, GROUP], F32, tag="pj", bufs=2)
                    for hq in range(4):
                        nc.tensor.matmul(
                            ps_pj,
                            lhsT=pjw_sb[
                                :, (e * 4 + hq) * 128 : (e * 4 + hq + 1) * 128
                            ],
                            rhs=h_sb[:, hq * GROUP : (hq + 1) * GROUP],
                            start=(hq == 0),
                            stop=(hq == 3),
                        )
                    pjT_sb = work.tile([128, GROUP], BF16, tag="pjT", bufs=18)
                    nc.vector.tensor_scalar_add(pjT_sb, ps_pj, pjb_sb[:, e : e + 1])
                    pjT.append(pjT_sb)

                # ---- pass 2: gate evac, transpose to [tok, e], threshold ----
                gsb = work.tile([16, GROUP], F32, tag="gsb", bufs=2)
                nc.vector.tensor_copy(gsb, ps_g)
                ps_gt = psum.tile([128, NTAU * E], F32, tag="tp", bufs=3)
                for ti in range(NTAU):
                    nc.tensor.transpose(
                        ps_gt[:, ti * E : (ti + 1) * E],
                        gsb[:, ti * 128 : (ti + 1) * 128],
                        idn_sb[:16, :16],
                    )
                gw = []
                for ti in range(NTAU):
                    gwt = work.tile([128, E], F32, tag="gw", bufs=8)
                    nc.vector.tensor_tensor(
                        gwt, ps_gt[:, ti * E : (ti + 1) * E], ngb_sb, ALU.is_gt
                    )
                    gw.append(gwt)

                out_sb = [
                    work.tile([128, C], F32, tag="out", bufs=4, name=f"osb_{g}_{ti}")
                    for ti in range(NTAU)
                ]

                # ---- pass 3: exit transposes + gated evacuation ----
                for e in range(E):
                    ps_o = psum.tile([128, GROUP], BF16, tag="tp", bufs=3)
                    for ti in range(NTAU):
                        nc.tensor.transpose(
                            ps_o[:, ti * 128 : (ti + 1) * 128],
                            pjT[e][:, ti * 128 : (ti + 1) * 128],
                            idnb_sb,
                        )
                    for ti in range(NTAU):
                        nc.vector.tensor_scalar_mul(
                            out_sb[ti][:, e * 128 : (e + 1) * 128],
                            ps_o[:, ti * 128 : (ti + 1) * 128],
                            gw[ti][:, e : e + 1],
                        )

                for ti in range(NTAU):
                    nc.sync.dma_start(
                        out=out_d[t0 + ti * 128 : t0 + (ti + 1) * 128, :],
                        in_=out_sb[ti],
                    )

    nc.compile()
    return nc


def _prep_inputs(x, gate_w, gate_b, fc_w, fc_b, proj_w, proj_b):
    x = np.ascontiguousarray(np.asarray(x, dtype=np.float32)).reshape(NTOK, C)
    gate_w = np.asarray(gate_w, dtype=np.float32)
    gate_b = np.asarray(gate_b, dtype=np.float32)
    fc_w = np.asarray(fc_w, dtype=np.float32)
    fc_b = np.asarray(fc_b, dtype=np.float32)
    proj_w = np.asarray(proj_w, dtype=np.float32)
    proj_b = np.asarray(proj_b, dtype=np.float32)

    # permuted channel order: c' = e*128 + d  ->  orig c = 16*d + e
    cp = np.arange(C)
    orig = 16 * (cp % DE) + cp // DE
    gwp = np.ascontiguousarray(gate_w[:, orig].T)  # [C, E]
    fcw = np.ascontiguousarray(fc_w.transpose(0, 2, 1))  # [E, DE, H]
    pjw = np.ascontiguousarray(
        proj_w.transpose(0, 2, 1).reshape(E, 4, 128, DE)
    )  # [E, q, h_in_chunk, d]
    fcb = np.ascontiguousarray(
        fc_b.reshape(E, 4, 128).transpose(2, 0, 1).reshape(128, E * 4)
    )
    pjb = np.ascontiguousarray(proj_b.T)  # [DE, E]
    ngb = np.ascontiguousarray(np.broadcast_to(-gate_b, (128, E)))
    idn = np.eye(128, dtype=np.float32)
    idnb = np.eye(128, dtype=np.float32).astype(ml_dtypes.bfloat16)

    shared = {
        "gwp": gwp,
        "fcw": fcw,
        "pjw": pjw,
        "fcb": fcb,
        "pjb": pjb,
        "ngb": ngb,
        "idn": idn,
        "idnb": idnb,
    }
    in_maps = [
        {"x": np.ascontiguousarray(x[i * TPC : (i + 1) * TPC]), **shared}
        for i in range(NCORES)
    ]
    return in_maps


def kernel(x, gate_w, gate_b, fc_w, fc_b, proj_w, proj_b, _trace=False, _tmpdir=None):
    if "nc" not in _CACHE:
        _CACHE["nc"] = _build()
    nc = _CACHE["nc"]
    in_maps = _prep_inputs(x, gate_w, gate_b, fc_w, fc_b, proj_w, proj_b)
    res = run_bass_kernel_spmd(
        nc,
        in_maps,
        core_ids=list(range(NCORES)),
        trace=_trace,
        tmpdir=_tmpdir,
    )
    out = np.concatenate([res.results[i]["out"] for i in range(NCORES)], axis=0)
    out = out.reshape(B, T, C)
    if _trace:
        _CACHE["last_result"] = res
    return out



# revision 4
# speedup vs baseline: 1.0034x; 1.0034x over previous
"""DynamicSparseMoE Trainium2 kernel.

Math (per token t):
  logits[e'] = x[t] . gate_w[e'] + gate_b[e']        (C=2048 contraction)
  gw[e']     = 1.0 if logits[e'] > 0 else 0.0
  expert e input: xe[d] = x[t, 16*d + e]  (d=0..127; expert idx fastest in channel)
  h  = gelu(fc_w[e] @ xe + fc_b[e])                   (H=512)
  oe = proj_w[e] @ h + proj_b[e]                      (DE=128)
  out[t, 128*e + d] = gw[e] * oe[d]                   (expert-major output channels)

Strategy: data-parallel over the 16384 tokens across 8 NeuronCores (2048
tokens/core).  Per 512-token group:
  per expert e: 4 fp32 PE transposes of the stride-16 channel slice ->
    xe^T [de, tok] fp32 in PSUM; DVE evacuates a bf16 hi part and a bf16
    lo (residual) part.  Gate logits accumulate in one PSUM bank as three
    bf16 matmul passes (w_hi*x_hi + w_hi*x_lo + w_lo*x_hi) -- exact enough
    for the sign decision since PSUM accumulates fp32.  fc/proj run as
    bf16 (weights bf16 stationary, x_hi / h bf16 moving, N=512 -> 1
    cyc/row).  gelu+fc_bias fused on ACT writing bf16 h, batched over
    [128, 1024] tiles when fc_b is all-zero.  proj bias fused into the
    PSUM->SBUF bf16 evacuation.
  gate: one is_gt tensor_scalar on the [16, 512] logits psum, 4 tiny PE
    transposes to token-major, one DVE evac.
  pass 3 per expert: 4 bf16 PE exit transposes; gate multiply fused into
    the final PSUM->SBUF copy as a per-token tensor_scalar; contiguous
    row stores.
"""

import sys

for _p in ("/opt/trn_rl_repo", "/root/.axon_site"):
    if _p not in sys.path:
        sys.path.insert(0, _p)

import ml_dtypes
import numpy as np

import concourse.mybir as mybir
from concourse import bacc
from concourse.bass_utils import run_bass_kernel_spmd
from concourse.tile import TileContext


B, T, C, E = 8, 2048, 2048, 16
DE = C // E  # 128
H = 4 * DE  # 512
NCORES = 8
NTOK = B * T  # 16384
TPC = NTOK // NCORES  # tokens per core: 2048
GROUP = 512  # tokens per group
NTAU = GROUP // 128  # 4 token-tiles per group
NGRP = TPC // GROUP  # 4 groups per core

F32 = mybir.dt.float32
BF16 = mybir.dt.bfloat16
AF = mybir.ActivationFunctionType
ALU = mybir.AluOpType
GELU = AF.Gelu

_CACHE = {}


def _build(fcb_zero):
    nc = bacc.Bacc(trn_type="TRN2", num_devices=NCORES)

    x_d = nc.dram_tensor("x", [TPC, C], F32, kind="ExternalInput").ap()
    gwh_d = nc.dram_tensor("gwh", [128, E * E], BF16, kind="ExternalInput").ap()
    gwl_d = nc.dram_tensor("gwl", [128, E * E], BF16, kind="ExternalInput").ap()
    fcw_d = nc.dram_tensor("fcw", [128, E * H], BF16, kind="ExternalInput").ap()
    pjw_d = nc.dram_tensor("pjw", [128, E * 4 * DE], BF16, kind="ExternalInput").ap()
    fcb_d = nc.dram_tensor("fcb", [128, E * 4], F32, kind="ExternalInput").ap()
    pjb_d = nc.dram_tensor("pjb", [128, E], F32, kind="ExternalInput").ap()
    ngb_d = nc.dram_tensor("ngb", [E, 1], F32, kind="ExternalInput").ap()
    idn_d = nc.dram_tensor("idn", [128, 128], F32, kind="ExternalInput").ap()
    idnb_d = nc.dram_tensor("idnb", [128, 128], BF16, kind="ExternalInput").ap()
    out_d = nc.dram_tensor("out", [TPC, C], F32, kind="ExternalOutput").ap()

    with TileContext(nc) as tc:
        with (
            tc.tile_pool(name="wts", bufs=1) as wts,
            tc.tile_pool(name="work", bufs=2) as work,
            tc.tile_pool(name="psum", bufs=2, space="PSUM") as psum,
        ):
            # ---- resident weights (all host-prepped dtypes; plain DMA) ----
            gwh_sb = wts.tile([128, E * E], BF16)  # [p, chunk*16+e']
            nc.sync.dma_start(out=gwh_sb, in_=gwh_d)
            gwl_sb = wts.tile([128, E * E], BF16)
            nc.sync.dma_start(out=gwl_sb, in_=gwl_d)
            fcw_sb = wts.tile([128, E * H], BF16)  # [de, e*512+h]
            nc.sync.dma_start(out=fcw_sb, in_=fcw_d)
            pjw_sb = wts.tile([128, E * 4 * DE], BF16)  # [h_in_chunk, (e*4+q)*128+d]
            nc.sync.dma_start(out=pjw_sb, in_=pjw_d)
            fcb_sb = wts.tile([128, E * 4], F32)
            nc.sync.dma_start(out=fcb_sb, in_=fcb_d)
            pjb_sb = wts.tile([128, E], F32)
            nc.sync.dma_start(out=pjb_sb, in_=pjb_d)
            ngb_sb = wts.tile([E, 1], F32)
            nc.sync.dma_start(out=ngb_sb, in_=ngb_d)
            idn_sb = wts.tile([128, 128], F32)
            nc.sync.dma_start(out=idn_sb, in_=idn_d)
            idnb_sb = wts.tile([128, 128], BF16)
            nc.sync.dma_start(out=idnb_sb, in_=idnb_d)

            for g in range(NGRP):
                t0 = g * GROUP
                xrow = []
                for ti in range(NTAU):
                    xt = work.tile([128, C], F32, tag="xrow", bufs=6)
                    nc.sync.dma_start(
                        out=xt, in_=x_d[t0 + ti * 128 : t0 + (ti + 1) * 128, :]
                    )
                    xrow.append(xt)

                ps_g = psum.tile([16, GROUP], F32, tag="gate", bufs=1)
                pjT = work.tile([128, E * GROUP], BF16, tag="pjT", bufs=2)
                # ---- pass 1: per-expert transpose, hi/lo, gate, fc, proj ----
                for e in range(E):
                    ps_t = psum.tile([128, GROUP], F32, tag="tp", bufs=2)
                    for ti in range(NTAU):
                        lhs = xrow[ti].rearrange("p (d e) -> p e d", e=E)[:, e, :]
                        nc.tensor.transpose(
                            ps_t[:, ti * 128 : (ti + 1) * 128], lhs, idn_sb
                        )
                    xh = work.tile([128, GROUP], BF16, tag="xh", bufs=3)
                    nc.vector.tensor_copy(xh, ps_t)
                    xl = work.tile([128, GROUP], BF16, tag="xl", bufs=3)
                    nc.vector.scalar_tensor_tensor(
                        xl, ps_t, 1.0, xh, ALU.mult, ALU.subtract
                    )
                    # gate: 3 bf16 passes, fp32 PSUM accumulation in one bank
                    nc.tensor.matmul(
                        ps_g,
                        lhsT=gwh_sb[:, e * E : (e + 1) * E],
                        rhs=xh,
                        start=(e == 0),
                        stop=False,
                    )
                    nc.tensor.matmul(
                        ps_g,
                        lhsT=gwh_sb[:, e * E : (e + 1) * E],
                        rhs=xl,
                        start=False,
                        stop=False,
                    )
                    nc.tensor.matmul(
                        ps_g,
                        lhsT=gwl_sb[:, e * E : (e + 1) * E],
                        rhs=xh,
                        start=False,
                        stop=(e == E - 1),
                    )

                    h_sb = work.tile([128, 4 * GROUP], BF16, tag="h", bufs=2)
                    if fcb_zero:
                        # two [128, 2*GROUP] fc psum tiles, one gelu each
                        for hh in range(2):
                            ps_fc = psum.tile([128, 2 * GROUP], F32, tag="fc", bufs=2)
                            for hq in (2 * hh, 2 * hh + 1):
                                nc.tensor.matmul(
                                    ps_fc[:, (hq % 2) * GROUP : (hq % 2 + 1) * GROUP],
                                    lhsT=fcw_sb[
                                        :, e * H + hq * 128 : e * H + (hq + 1) * 128
                                    ],
                                    rhs=xh,
                                    start=True,
                                    stop=True,
                                )
                            nc.scalar.activation(
                                h_sb[:, 2 * hh * GROUP : 2 * (hh + 1) * GROUP],
                                ps_fc,
                                GELU,
                                scale=1.0,
                            )
                    else:
                        for hh in range(2):
                            ps_fc = psum.tile([128, 2 * GROUP], F32, tag="fc", bufs=2)
                            for hq in (2 * hh, 2 * hh + 1):
                                nc.tensor.matmul(
                                    ps_fc[:, (hq % 2) * GROUP : (hq % 2 + 1) * GROUP],
                                    lhsT=fcw_sb[
                                        :, e * H + hq * 128 : e * H + (hq + 1) * 128
                                    ],
                                    rhs=xh,
                                    start=True,
                                    stop=True,
                                )
                            for hq in (2 * hh, 2 * hh + 1):
                                nc.scalar.activation(
                                    h_sb[:, hq * GROUP : (hq + 1) * GROUP],
                                    ps_fc[:, (hq % 2) * GROUP : (hq % 2 + 1) * GROUP],
                                    GELU,
                                    bias=fcb_sb[:, e * 4 + hq : e * 4 + hq + 1],
                                    scale=1.0,
                                )

                    ps_pj = psum.tile([128, GROUP], F32, tag="pj", bufs=1)
                    for hq in range(4):
                        nc.tensor.matmul(
                            ps_pj,
                            lhsT=pjw_sb[
                                :, (e * 4 + hq) * 128 : (e * 4 + hq + 1) * 128
                            ],
                            rhs=h_sb[:, hq * GROUP : (hq + 1) * GROUP],
                            start=(hq == 0),
                            stop=(hq == 3),
                        )
                    nc.vector.tensor_scalar_add(
                        pjT[:, e * GROUP : (e + 1) * GROUP],
                        ps_pj,
                        pjb_sb[:, e : e + 1],
                    )

                # ---- pass 2: gate threshold + transpose to [tok, e] ----
                gwf = work.tile([16, GROUP], BF16, tag="gwf", bufs=2)
                nc.vector.tensor_scalar(
                    gwf, ps_g, ngb_sb, None, ALU.is_gt
                )
                ps_gt = psum.tile([128, NTAU * E], BF16, tag="tp", bufs=2)
                for ti in range(NTAU):
                    nc.tensor.transpose(
                        ps_gt[:, ti * E : (ti + 1) * E],
                        gwf[:, ti * 128 : (ti + 1) * 128],
                        idnb_sb[:16, :16],
                    )
                gw_sb = work.tile([128, NTAU * E], F32, tag="gw", bufs=2)
                nc.vector.tensor_copy(gw_sb, ps_gt)

                out_sb = [
                    work.tile([128, C], F32, tag="out", bufs=6, name=f"osb_{g}_{ti}")
                    for ti in range(NTAU)
                ]

                # ---- pass 3: exit transposes + gated evacuation ----
                for e in range(E):
                    ps_o = psum.tile([128, GROUP], BF16, tag="tp", bufs=2)
                    for ti in range(NTAU):
                        nc.tensor.transpose(
                            ps_o[:, ti * 128 : (ti + 1) * 128],
                            pjT[:, e * GROUP + ti * 128 : e * GROUP + (ti + 1) * 128],
                            idnb_sb,
                        )
                    for ti in range(NTAU):
                        nc.vector.tensor_scalar_mul(
                            out_sb[ti][:, e * 128 : (e + 1) * 128],
                            ps_o[:, ti * 128 : (ti + 1) * 128],
                            gw_sb[:, ti * E + e : ti * E + e + 1],
                        )

                for ti in range(NTAU):
                    nc.sync.dma_start(
                        out=out_d[t0 + ti * 128 : t0 + (ti + 1) * 128, :],
                        in_=out_sb[ti],
                    )

    nc.compile()
    return nc


def _prep_inputs(x, gate_w, gate_b, fc_w, fc_b, proj_w, proj_b):
    bf16 = ml_dtypes.bfloat16
    x = np.ascontiguousarray(np.asarray(x, dtype=np.float32)).reshape(NTOK, C)
    gate_w = np.asarray(gate_w, dtype=np.float32)
    gate_b = np.asarray(gate_b, dtype=np.float32)
    fc_w = np.asarray(fc_w, dtype=np.float32)
    fc_b = np.asarray(fc_b, dtype=np.float32)
    proj_w = np.asarray(proj_w, dtype=np.float32)
    proj_b = np.asarray(proj_b, dtype=np.float32)

    # permuted channel order: c' = e*128 + d  ->  orig c = 16*d + e
    cp = np.arange(C)
    orig = 16 * (cp % DE) + cp // DE
    gwp = np.ascontiguousarray(gate_w[:, orig].T)  # [C, E] fp32, rows = c'
    gwp_hi = gwp.astype(bf16)
    gwp_lo = (gwp - gwp_hi.astype(np.float32)).astype(bf16)
    # -> [p, chunk*16+e]
    gwh = np.ascontiguousarray(
        gwp_hi.reshape(E, 128, E).transpose(1, 0, 2).reshape(128, E * E)
    )
    gwl = np.ascontiguousarray(
        gwp_lo.reshape(E, 128, E).transpose(1, 0, 2).reshape(128, E * E)
    )
    # fc weights: [E, H, DE] -> [de, e*512+h] bf16
    fcw = np.ascontiguousarray(
        fc_w.transpose(2, 0, 1).reshape(128, E * H)
    ).astype(bf16)
    # proj weights: [E, DE, H] -> [h_in_chunk, (e*4+q)*128+d] bf16
    pjw = np.ascontiguousarray(
        proj_w.transpose(0, 2, 1).reshape(E, 4, 128, DE).transpose(2, 0, 1, 3)
        .reshape(128, E * 4 * DE)
    ).astype(bf16)
    fcb = np.ascontiguousarray(
        fc_b.reshape(E, 4, 128).transpose(2, 0, 1).reshape(128, E * 4)
    )
    pjb = np.ascontiguousarray(proj_b.reshape(E, DE).T)  # [DE, E]
    ngb = np.ascontiguousarray((-gate_b).reshape(E, 1))
    idn = np.eye(128, dtype=np.float32)
    idnb = np.eye(128, dtype=np.float32).astype(bf16)

    fcb_zero = not np.any(fc_b)

    shared = {
        "gwh": gwh,
        "gwl": gwl,
        "fcw": fcw,
        "pjw": pjw,
        "fcb": fcb,
        "pjb": pjb,
        "ngb": ngb,
        "idn": idn,
        "idnb": idnb,
    }
    in_maps = [
        {"x": np.ascontiguousarray(x[i * TPC : (i + 1) * TPC]), **shared}
        for i in range(NCORES)
    ]
    return in_maps, fcb_zero


def kernel(x, gate_w, gate_b, fc_w, fc_b, proj_w, proj_b, _trace=False, _tmpdir=None):
    in_maps, fcb_zero = _prep_inputs(
        x, gate_w, gate_b, fc_w, fc_b, proj_w, proj_b
    )
    key = ("nc", fcb_zero)
    if key not in _CACHE:
        _CACHE[key] = _build(fcb_zero)
    nc = _CACHE[key]
    res = run_bass_kernel_spmd(
        nc,
        in_maps,
        core_ids=list(range(NCORES)),
        trace=_trace,
        tmpdir=_tmpdir,
    )
    out = np.concatenate([res.results[i]["out"] for i in range(NCORES)], axis=0)
    out = out.reshape(B, T, C)
    if _trace:
        _CACHE["last_result"] = res
    return out


# revision 7
# speedup vs baseline: 1.2075x; 1.2035x over previous
"""DynamicSparseMoE Trainium2 kernel.

Math (per token t):
  logits[e'] = x[t] . gate_w[e'] + gate_b[e']        (C=2048 contraction)
  gw[e']     = 1.0 if logits[e'] > 0 else 0.0
  expert e input: xe[d] = x[t, 16*d + e]  (d=0..127; expert idx fastest in channel)
  h  = gelu(fc_w[e] @ xe + fc_b[e])                   (H=512)
  oe = proj_w[e] @ h + proj_b[e]                      (DE=128)
  out[t, 128*e + d] = gw[e] * oe[d]                   (expert-major output channels)

Strategy: data-parallel over the 16384 tokens across 8 NeuronCores (2048
tokens/core, processed as 4 groups x 16 experts = 64 flat iterations).

All engines are in-order, so the kernel is emitted as an explicit
software pipeline over the flat iteration index k with per-stage lags:

  PE  iter k: entry-transposes(k) | gate-mms(k-1) | fc-mms(k-1) |
              proj-mms(k-2) | exit-transposes(k-18) | [gate-T at group end]
  DVE iter k: x_hi evac(k), x_lo evac(k) | [gw copy / is_gt at group end]
              | exit gated evac ti 0,1 (k-18) | pjT evac(k-2)
  ACT iter k: exit gated evac ti 2,3 (k-18) | gelu(k-1)

Numerics: entry transposes are fp32 (PE, exact); DVE evacuates a bf16 hi
part and bf16 lo residual.  Gate logits accumulate in one PSUM bank over
48 bf16 matmuls (w_hi*x_hi + w_hi*x_lo + w_lo*x_hi), exact enough for the
sign decision.  fc/proj are bf16 x bf16 with fp32 PSUM accumulation.
gelu on ACT writes bf16 h ([128,1024] tiles when fc_b==0).  proj bias is
fused into the PSUM->SBUF bf16 evac; the gate multiply is fused into the
final PSUM->SBUF fp32 evac (tensor_scalar on DVE / scale-Copy on ACT).
"""

import sys

for _p in ("/opt/trn_rl_repo", "/root/.axon_site"):
    if _p not in sys.path:
        sys.path.insert(0, _p)

import ml_dtypes
import numpy as np

import concourse.mybir as mybir
from concourse import bacc
from concourse.bass_utils import run_bass_kernel_spmd
from concourse.tile import TileContext


B, T, C, E = 8, 2048, 2048, 16
DE = C // E  # 128
H = 4 * DE  # 512
NCORES = 8
NTOK = B * T  # 16384
TPC = NTOK // NCORES  # tokens per core: 2048
GROUP = 512  # tokens per group
NTAU = GROUP // 128  # 4 token-tiles per group
NGRP = TPC // GROUP  # 4 groups per core
NS = NGRP * E  # 64 flat iterations

LAG_FC = 1  # gate+fc consume xh/xl one iter after the transpose
LAG_PJ = 2  # proj consumes h one further iter later
LAG_EX = 18  # exit transposes/evacs trail by 18 (gw ready after group+2)

F32 = mybir.dt.float32
BF16 = mybir.dt.bfloat16
AF = mybir.ActivationFunctionType
ALU = mybir.AluOpType
GELU = AF.Gelu

_CACHE = {}


def _build(fcb_zero):
    nc = bacc.Bacc(trn_type="TRN2", num_devices=NCORES)

    x_d = nc.dram_tensor("x", [TPC, C], F32, kind="ExternalInput").ap()
    gwh_d = nc.dram_tensor("gwh", [128, E * E], BF16, kind="ExternalInput").ap()
    gwl_d = nc.dram_tensor("gwl", [128, E * E], BF16, kind="ExternalInput").ap()
    fcw_d = nc.dram_tensor("fcw", [128, E * H], BF16, kind="ExternalInput").ap()
    pjw_d = nc.dram_tensor("pjw", [128, E * 4 * DE], BF16, kind="ExternalInput").ap()
    fcb_d = nc.dram_tensor("fcb", [128, E * 4], F32, kind="ExternalInput").ap()
    pjb_d = nc.dram_tensor("pjb", [128, E], F32, kind="ExternalInput").ap()
    ngb_d = nc.dram_tensor("ngb", [E, 1], F32, kind="ExternalInput").ap()
    idn_d = nc.dram_tensor("idn", [128, 128], F32, kind="ExternalInput").ap()
    idnb_d = nc.dram_tensor("idnb", [128, 128], BF16, kind="ExternalInput").ap()
    out_d = nc.dram_tensor("out", [TPC, C], F32, kind="ExternalOutput").ap()

    with TileContext(nc) as tc:
        with (
            tc.tile_pool(name="wts", bufs=1) as wts,
            tc.tile_pool(name="work", bufs=2) as work,
            tc.tile_pool(name="psum", bufs=2, space="PSUM") as psum,
        ):
            # ---- resident weights (all host-prepped dtypes; plain DMA) ----
            gwh_sb = wts.tile([128, E * E], BF16)
            nc.sync.dma_start(out=gwh_sb, in_=gwh_d)
            gwl_sb = wts.tile([128, E * E], BF16)
            nc.sync.dma_start(out=gwl_sb, in_=gwl_d)
            fcw_sb = wts.tile([128, E * H], BF16)
            nc.sync.dma_start(out=fcw_sb, in_=fcw_d)
            pjw_sb = wts.tile([128, E * 4 * DE], BF16)
            nc.sync.dma_start(out=pjw_sb, in_=pjw_d)
            fcb_sb = wts.tile([128, E * 4], F32)
            nc.sync.dma_start(out=fcb_sb, in_=fcb_d)
            pjb_sb = wts.tile([128, E], F32)
            nc.sync.dma_start(out=pjb_sb, in_=pjb_d)
            ngb_sb = wts.tile([E, 1], F32)
            nc.sync.dma_start(out=ngb_sb, in_=ngb_d)
            idn_sb = wts.tile([128, 128], F32)
            nc.sync.dma_start(out=idn_sb, in_=idn_d)
            idnb_sb = wts.tile([128, 128], BF16)
            nc.sync.dma_start(out=idnb_sb, in_=idnb_d)

            # pipeline state, keyed by flat iteration / group
            xrow = {}  # g -> [4 row tiles]
            psg = {}  # g -> gate psum
            pst = {}  # s -> entry-transpose psum
            xhs = {}  # s -> x hi bf16
            xls = {}  # s -> x lo bf16
            fcp = {}  # s -> [2 fc psum tiles]
            hsb = {}  # s -> h bf16
            pjp = {}  # s -> proj psum
            pjT = {}  # g -> [128, E*GROUP] bf16
            gwf = {}  # g -> [16, GROUP] bf16 gate one-hot (feature-major)
            psgt = {}  # g -> gate transpose psum
            gwsb = {}  # g -> [128, NTAU*E] f32 gate (token-major)
            outsb = {}  # g -> [4 out tiles]
            exps = {}  # s -> exit transpose psum

            def dma_loads(g):
                xrow[g] = []
                for ti in range(NTAU):
                    xt = work.tile([128, C], F32, tag="xrow", bufs=8,
                                   name=f"xr_{g}_{ti}")
                    nc.sync.dma_start(
                        out=xt,
                        in_=x_d[g * GROUP + ti * 128 : g * GROUP + (ti + 1) * 128, :],
                    )
                    xrow[g].append(xt)

            def pe_tin(s):
                g = s // E
                e = s % E
                ps_t = psum.tile([128, GROUP], F32, tag="tp", bufs=2, name=f"t_{s}")
                for ti in range(NTAU):
                    lhs = xrow[g][ti].rearrange("p (d e) -> p e d", e=E)[:, e, :]
                    nc.tensor.transpose(
                        ps_t[:, ti * 128 : (ti + 1) * 128], lhs, idn_sb
                    )
                pst[s] = ps_t
                if e == E - 1:
                    del xrow[g]

            def pe_gate(s):
                g = s // E
                e = s % E
                if e == 0:
                    psg[g] = psum.tile([16, GROUP], F32, tag="gate", bufs=1,
                                       name=f"g_{g}")
                ps_g = psg[g]
                xh, xl = xhs[s], xls[s]
                nc.tensor.matmul(
                    ps_g, lhsT=gwh_sb[:, e * E : (e + 1) * E], rhs=xh,
                    start=(e == 0), stop=False,
                )
                nc.tensor.matmul(
                    ps_g, lhsT=gwh_sb[:, e * E : (e + 1) * E], rhs=xl,
                    start=False, stop=False,
                )
                nc.tensor.matmul(
                    ps_g, lhsT=gwl_sb[:, e * E : (e + 1) * E], rhs=xh,
                    start=False, stop=(e == E - 1),
                )

            def pe_fc(s):
                e = s % E
                xh = xhs[s]
                tiles = []
                for hh in range(2):
                    ps_fc = psum.tile([128, 2 * GROUP], F32, tag="fc", bufs=2,
                                      name=f"fc_{s}_{hh}")
                    for hq in (2 * hh, 2 * hh + 1):
                        nc.tensor.matmul(
                            ps_fc[:, (hq % 2) * GROUP : (hq % 2 + 1) * GROUP],
                            lhsT=fcw_sb[:, e * H + hq * 128 : e * H + (hq + 1) * 128],
                            rhs=xh,
                            start=True, stop=True,
                        )
                    tiles.append(ps_fc)
                fcp[s] = tiles

            def pe_proj(s):
                g = s // E
                e = s % E
                h_sb = hsb[s]
                ps_pj = psum.tile([128, GROUP], F32, tag="pj", bufs=1, name=f"pj_{s}")
                for hq in range(4):
                    nc.tensor.matmul(
                        ps_pj,
                        lhsT=pjw_sb[:, (e * 4 + hq) * 128 : (e * 4 + hq + 1) * 128],
                        rhs=h_sb[:, hq * GROUP : (hq + 1) * GROUP],
                        start=(hq == 0), stop=(hq == 3),
                    )
                pjp[s] = ps_pj
                if e == 0:
                    pjT[g] = work.tile([128, E * GROUP], BF16, tag="pjT", bufs=2,
                                       name=f"pjT_{g}")

            def pe_exit(s):
                g = s // E
                e = s % E
                ps_o = psum.tile([128, GROUP], BF16, tag="tp", bufs=2, name=f"x_{s}")
                for ti in range(NTAU):
                    nc.tensor.transpose(
                        ps_o[:, ti * 128 : (ti + 1) * 128],
                        pjT[g][:, e * GROUP + ti * 128 : e * GROUP + (ti + 1) * 128],
                        idnb_sb,
                    )
                exps[s] = ps_o

            def pe_gate_t(g):
                ps_gt = psum.tile([128, NTAU * E], BF16, tag="tp", bufs=2,
                                  name=f"gt_{g}")
                for ti in range(NTAU):
                    nc.tensor.transpose(
                        ps_gt[:, ti * E : (ti + 1) * E],
                        gwf[g][:, ti * 128 : (ti + 1) * 128],
                        idnb_sb[:16, :16],
                    )
                psgt[g] = ps_gt

            def dve_evac_x(s):
                ps_t = pst.pop(s)
                xh = work.tile([128, GROUP], BF16, tag="xh", bufs=3, name=f"xh_{s}")
                nc.vector.tensor_copy(xh, ps_t)
                xl = work.tile([128, GROUP], BF16, tag="xl", bufs=3, name=f"xl_{s}")
                nc.vector.scalar_tensor_tensor(xl, ps_t, 1.0, xh, ALU.mult,
                                               ALU.subtract)
                xhs[s], xls[s] = xh, xl

            def dve_gwf(g):
                t = work.tile([16, GROUP], BF16, tag="gwf", bufs=2, name=f"gwf_{g}")
                nc.vector.tensor_scalar(t, psg.pop(g), ngb_sb, None, ALU.is_gt)
                gwf[g] = t

            def dve_gw(g):
                t = work.tile([128, NTAU * E], F32, tag="gw", bufs=2, name=f"gw_{g}")
                nc.vector.tensor_copy(t, psgt.pop(g))
                gwsb[g] = t
                del gwf[g]

            def dve_pjt(s):
                g = s // E
                e = s % E
                nc.vector.tensor_scalar_add(
                    pjT[g][:, e * GROUP : (e + 1) * GROUP],
                    pjp.pop(s),
                    pjb_sb[:, e : e + 1],
                )

            def evac_exit(s):
                g = s // E
                e = s % E
                if e == 0:
                    outsb[g] = [
                        work.tile([128, C], F32, tag="out", bufs=6,
                                  name=f"o_{g}_{ti}")
                        for ti in range(NTAU)
                    ]
                ps_o = exps.pop(s)
                gw = gwsb[g]
                for ti in range(2):
                    nc.vector.tensor_scalar_mul(
                        outsb[g][ti][:, e * 128 : (e + 1) * 128],
                        ps_o[:, ti * 128 : (ti + 1) * 128],
                        gw[:, ti * E + e : ti * E + e + 1],
                    )
                for ti in range(2, NTAU):
                    nc.scalar.activation(
                        outsb[g][ti][:, e * 128 : (e + 1) * 128],
                        ps_o[:, ti * 128 : (ti + 1) * 128],
                        AF.Copy,
                        scale=gw[:, ti * E + e : ti * E + e + 1],
                    )

            def act_gelu(s):
                e = s % E
                h_sb = work.tile([128, 4 * GROUP], BF16, tag="h", bufs=3,
                                 name=f"h_{s}")
                tiles = fcp.pop(s)
                for hh in range(2):
                    if fcb_zero:
                        nc.scalar.activation(
                            h_sb[:, 2 * hh * GROUP : 2 * (hh + 1) * GROUP],
                            tiles[hh], GELU, scale=1.0,
                        )
                    else:
                        for hq in (2 * hh, 2 * hh + 1):
                            nc.scalar.activation(
                                h_sb[:, hq * GROUP : (hq + 1) * GROUP],
                                tiles[hh][:, (hq % 2) * GROUP : (hq % 2 + 1) * GROUP],
                                GELU,
                                bias=fcb_sb[:, e * 4 + hq : e * 4 + hq + 1],
                                scale=1.0,
                            )
                hsb[s] = h_sb

            def dma_out(g):
                for ti in range(NTAU):
                    nc.sync.dma_start(
                        out=out_d[g * GROUP + ti * 128 : g * GROUP + (ti + 1) * 128, :],
                        in_=outsb[g][ti],
                    )
                del outsb[g]
                del gwsb[g]
                del pjT[g]

            # ---------------- the pipeline ----------------
            for k in range(NS + LAG_EX + 1):
                s_tin = k
                s_fc = k - LAG_FC
                s_pj = k - LAG_PJ
                s_ex = k - LAG_EX

                # --- PE stream ---
                if s_tin < NS:
                    if s_tin == 0:
                        dma_loads(0)
                    pe_tin(s_tin)
                if s_tin < NS and s_tin % E == 10 and s_tin // E + 1 < NGRP:
                    dma_loads(s_tin // E + 1)
                if 0 <= s_fc < NS:
                    pe_gate(s_fc)
                    pe_fc(s_fc)
                if 0 <= s_pj < NS:
                    pe_proj(s_pj)
                if 0 <= s_ex < NS:
                    pe_exit(s_ex)

                # --- DVE stream ---
                if 0 <= s_fc - 1 and (s_fc - 1) % E == E - 1 and s_fc - 1 < NS:
                    # gw copy for group finished two iters ago (ps_gt from
                    # last iter's PE tail)
                    dve_gw((s_fc - 1) // E)
                if s_tin < NS:
                    dve_evac_x(s_tin)
                if 0 <= s_fc < NS and s_fc % E == E - 1:
                    dve_gwf(s_fc // E)
                if 0 <= s_ex < NS:
                    evac_exit(s_ex)  # DVE ti 0,1 + ACT ti 2,3
                if 0 <= s_pj < NS:
                    dve_pjt(s_pj)

                # --- PE gate transpose (after gwf) ---
                if 0 <= s_fc < NS and s_fc % E == E - 1:
                    pe_gate_t(s_fc // E)

                # --- ACT stream ---
                if 0 <= s_fc < NS:
                    act_gelu(s_fc)

                # --- output DMA ---
                if 0 <= s_ex < NS and s_ex % E == E - 1:
                    dma_out(s_ex // E)

    nc.compile()
    return nc


def _prep_inputs(x, gate_w, gate_b, fc_w, fc_b, proj_w, proj_b):
    bf16 = ml_dtypes.bfloat16
    x = np.ascontiguousarray(np.asarray(x, dtype=np.float32)).reshape(NTOK, C)
    gate_w = np.asarray(gate_w, dtype=np.float32)
    gate_b = np.asarray(gate_b, dtype=np.float32)
    fc_w = np.asarray(fc_w, dtype=np.float32)
    fc_b = np.asarray(fc_b, dtype=np.float32)
    proj_w = np.asarray(proj_w, dtype=np.float32)
    proj_b = np.asarray(proj_b, dtype=np.float32)

    # permuted channel order: c' = e*128 + d  ->  orig c = 16*d + e
    cp = np.arange(C)
    orig = 16 * (cp % DE) + cp // DE
    gwp = np.ascontiguousarray(gate_w[:, orig].T)  # [C, E] fp32, rows = c'
    gwp_hi = gwp.astype(bf16)
    gwp_lo = (gwp - gwp_hi.astype(np.float32)).astype(bf16)
    gwh = np.ascontiguousarray(
        gwp_hi.reshape(E, 128, E).transpose(1, 0, 2).reshape(128, E * E)
    )
    gwl = np.ascontiguousarray(
        gwp_lo.reshape(E, 128, E).transpose(1, 0, 2).reshape(128, E * E)
    )
    fcw = np.ascontiguousarray(
        fc_w.transpose(2, 0, 1).reshape(128, E * H)
    ).astype(bf16)
    pjw = np.ascontiguousarray(
        proj_w.transpose(0, 2, 1).reshape(E, 4, 128, DE).transpose(2, 0, 1, 3)
        .reshape(128, E * 4 * DE)
    ).astype(bf16)
    fcb = np.ascontiguousarray(
        fc_b.reshape(E, 4, 128).transpose(2, 0, 1).reshape(128, E * 4)
    )
    pjb = np.ascontiguousarray(proj_b.reshape(E, DE).T)  # [DE, E]
    ngb = np.ascontiguousarray((-gate_b).reshape(E, 1))
    idn = np.eye(128, dtype=np.float32)
    idnb = np.eye(128, dtype=np.float32).astype(bf16)

    fcb_zero = not np.any(fc_b)

    shared = {
        "gwh": gwh,
        "gwl": gwl,
        "fcw": fcw,
        "pjw": pjw,
        "fcb": fcb,
        "pjb": pjb,
        "ngb": ngb,
        "idn": idn,
        "idnb": idnb,
    }
    in_maps = [
        {"x": np.ascontiguousarray(x[i * TPC : (i + 1) * TPC]), **shared}
        for i in range(NCORES)
    ]
    return in_maps, fcb_zero


def kernel(x, gate_w, gate_b, fc_w, fc_b, proj_w, proj_b, _trace=False, _tmpdir=None):
    in_maps, fcb_zero = _prep_inputs(
        x, gate_w, gate_b, fc_w, fc_b, proj_w, proj_b
    )
    key = ("nc", fcb_zero)
    if key not in _CACHE:
        _CACHE[key] = _build(fcb_zero)
    nc = _CACHE[key]
    res = run_bass_kernel_spmd(
        nc,
        in_maps,
        core_ids=list(range(NCORES)),
        trace=_trace,
        tmpdir=_tmpdir,
    )
    out = np.concatenate([res.results[i]["out"] for i in range(NCORES)], axis=0)
    out = out.reshape(B, T, C)
    if _trace:
        _CACHE["last_result"] = res
    return out
